# revision 4
# baseline (speedup 1.0000x reference)
"""Debayer 3x3 kernel for Trainium2 (Bass/Tile), batch-sharded over 8 NeuronCores.

Reference semantics: 1->5 channel 3x3 conv (identity, plus-4, diag-4,
horiz-2, vert-2) over an edge-padded Bayer frame, then per-2x2-parity
channel select into RGB.

v2 (memory-optimized): all device I/O in fp16 (tolerance is 2e-2; fp16
adds <1e-3), and the identity channel (1 of every 3 output values equals
the input pixel exactly) is pasted on the host from the original f32
input. The device computes only the 8 non-trivial quarter-resolution
planes per tile, packed contiguously for one large DMA per slice:
  P0 c1_ee->G  P1 c1_oo->G  P2 c2_ee->B  P3 c2_oo->R
  P4 c3_eo->R  P5 c3_oe->B  P6 c4_eo->B  P7 c4_oe->R
Per-pixel, with q = x/4:
  SQ[r,c] = q[r,c]+q[r,c+2]   (horiz pair, centered at out col c)
  VQ[r,c] = q[r,c]+q[r+2,c]   (vert pair, centered at out row r)
  c1 = SQ+VQ   c2 = SQ[up]+SQ[down]   c3 = 2*SQ   c4 = 2*VQ

Device traffic per core: in 128*4*36*122*2B = 4.5 MB, out
128*4*8*17*60*2B = 4.2 MB (vs 34 MB for the f32 3-channel baseline).
Full-res pair sums run on DVE in 2x packed-fp16 mode (step-1, 4B-aligned
APs); the stride-2 parity combines go to gpsimd (adds) and the scalar
activation engine (scaled copies), keeping DVE under the DMA roofline.

Device layout: the host pre-tiles each padded 1090x1922 fp16 image into
128 partitions x 4 col-slices x (36 rows x 122 cols) patches:
  partition p = 32*q + b  (col-quarter q in 0..3, row-band b in 0..31)
  band b   -> image rows [34b, 34b+34)        (patch has +-1 halo rows)
  slice s  -> image cols [480q+120s, +120)    (patch has +-1 halo cols)
34 and 120 are even so parity phase is uniform across partitions/slices.
"""

import numpy as np

H, W = 1088, 1920
NB = 32          # row bands per column-quarter
BH = 34          # output rows per band
NQ = 4           # column quarters
NS = 4           # col slices per patch
SW = 120         # output cols per slice
PR, PC = BH + 2, SW + 2   # patch rows/cols (with halo)
QR, QC = 17, 60           # quarter-res plane dims per tile

# (plane, channel, row parity, col parity) for host-side assembly
PLANES = [
    (0, 1, 0, 0),  # c1_ee -> G
    (1, 1, 1, 1),  # c1_oo -> G
    (2, 2, 0, 0),  # c2_ee -> B
    (3, 0, 1, 1),  # c2_oo -> R
    (4, 0, 0, 1),  # c3_eo -> R
    (5, 2, 1, 0),  # c3_oe -> B
    (6, 2, 0, 1),  # c4_eo -> B
    (7, 0, 1, 0),  # c4_oe -> R
]

_NC_CACHE = {}
LAST_RESULTS = None


def _build(reps=1, *, c12="gpsimd", sc="scalar", in_bufs=3, mid_bufs=2,
           out_bufs=2, skeleton=False, **_ignored):
    """Build the Bass module. reps>1 repeats the whole pipeline (bench only:
    amortizes per-dispatch overhead out of wall-clock measurements)."""
    key = (reps, c12, sc, in_bufs, mid_bufs, out_bufs, skeleton)
    if key in _NC_CACHE:
        return _NC_CACHE[key]
    import concourse.bacc as bacc
    import concourse.mybir as mybir
    import concourse.tile as tile
    from concourse._compat import get_trn_type

    f16 = mybir.dt.float16
    nc = bacc.Bacc(get_trn_type() or "TRN2", target_bir_lowering=False, debug=False)
    xin = nc.dram_tensor("xprep", [128, NS, PR, PC], f16, kind="ExternalInput")
    yout = nc.dram_tensor("yout", [128, NS, 8, QR, QC], f16, kind="ExternalOutput")
    # bench-only: earlier reps dump to internal scratch so no two reps write
    # the same DRAM (WAW races hang the exec unit)
    ydumps = [
        nc.dram_tensor(f"ydump{r}", [128, NS, 8, QR, QC], f16, kind="Internal")
        for r in range(reps - 1)
    ]

    with tile.TileContext(nc) as tc:
        with tc.tile_pool(name="pin", bufs=in_bufs) as pin, \
             tc.tile_pool(name="pmid", bufs=mid_bufs) as pmid, \
             tc.tile_pool(name="pout", bufs=out_bufs) as pout:

            eng12 = {"gpsimd": nc.gpsimd, "vector": nc.vector}[c12]
            if sc == "scalar":
                def smul(out, in_, s):
                    nc.scalar.mul(out, in_, s)
            else:
                eng_s = {"gpsimd": nc.gpsimd, "vector": nc.vector}[sc]
                def smul(out, in_, s):
                    eng_s.tensor_scalar_mul(out, in_, s)

            def load(j):
                t = pin.tile([128, PR, PC], f16, tag="inp", name=f"inp{j}")
                nc.sync.dma_start(out=t[:], in_=xin[:, j % NS])
                return t

            cur = load(0)
            for j in range(NS * reps):
                k = j % NS
                r = j // NS
                ytgt = yout if r == reps - 1 else ydumps[r]
                nxt = load(j + 1) if j + 1 < NS * reps else None
                Q = cur
                if skeleton:
                    # bench-only: DMA floor (touch input once so it's live)
                    Ysk = pout.tile([128, 8, QR, QC], f16, tag="y", name=f"y{k}")
                    nc.vector.tensor_copy(Ysk[:, 0, 0:1, :], Q[:, 0:1, 0:QC])
                    nc.sync.dma_start(out=ytgt[:, k], in_=Ysk[:])
                    cur = nxt
                    continue
                # prescale in place: Q = x/4 (TS, 4x packed mode)
                nc.vector.tensor_scalar_mul(Q[:], Q[:], 0.25)
                # SQ[p, r, c] = H/4 centered at out col c (TT, 2x mode)
                SQ = pmid.tile([128, PR, SW], f16, tag="sq", name=f"sq{k}")
                nc.vector.tensor_add(SQ[:], Q[:, :, 0:SW], Q[:, :, 2:PC])
                # VQ[p, r, c] = V/4 centered at out row r, full patch width
                VQ = pmid.tile([128, PR - 2, PC], f16, tag="vq", name=f"vq{k}")
                nc.vector.tensor_add(VQ[:], Q[:, 0:PR - 2, :], Q[:, 2:PR, :])

                Y = pout.tile([128, 8, QR, QC], f16, tag="y", name=f"y{k}")
                # c1 = SQ+VQ, c2 = SQ up+down (stride-2 parity reads)
                eng12.tensor_add(Y[:, 0], SQ[:, 1:35:2, 0:120:2], VQ[:, 0:34:2, 1:121:2])
                eng12.tensor_add(Y[:, 1], SQ[:, 2:36:2, 1:120:2], VQ[:, 1:34:2, 2:122:2])
                eng12.tensor_add(Y[:, 2], SQ[:, 0:34:2, 0:120:2], SQ[:, 2:36:2, 0:120:2])
                eng12.tensor_add(Y[:, 3], SQ[:, 1:35:2, 1:120:2], SQ[:, 3:36:2, 1:120:2])
                # c3 = 2*SQ, c4 = 2*VQ (scaled copies on the act engine)
                smul(Y[:, 4], SQ[:, 1:35:2, 1:120:2], 2.0)
                smul(Y[:, 5], SQ[:, 2:36:2, 0:120:2], 2.0)
                smul(Y[:, 6], VQ[:, 0:34:2, 2:122:2], 2.0)
                smul(Y[:, 7], VQ[:, 1:34:2, 1:121:2], 2.0)
                nc.sync.dma_start(out=ytgt[:, k], in_=Y[:])

                cur = nxt

    nc.compile()
    _NC_CACHE[key] = nc
    return nc


def _prep_inputs(x):
    """(B,1,1088,1920) f32 -> (B,128,NS,PR,PC) fp16 patch layout (edge padded)."""
    Bn = x.shape[0]
    xpad = np.pad(x[:, 0], ((0, 0), (1, 1), (1, 1)), mode="edge").astype(np.float16)
    xprep = np.empty((Bn, 128, NS, PR, PC), np.float16)
    st = xpad.strides
    for q in range(NQ):
        for s in range(NS):
            c0 = 480 * q + SW * s
            block = xpad[:, :, c0:c0 + PC]
            v = np.lib.stride_tricks.as_strided(
                block, shape=(Bn, NB, PR, PC),
                strides=(st[0], BH * st[1], st[1], st[2]))
            xprep[:, q * NB:(q + 1) * NB, s] = v
    return xprep


def _assemble(y, x):
    """y (B,128,NS,8,QR,QC) fp16 planes + x (B,1,H,W) f32 -> (B,3,H,W) f32."""
    Bn = x.shape[0]
    out = np.empty((Bn, 3, H, W), np.float32)
    # identity channels from the exact f32 input
    out[:, 0, 0::2, 0::2] = x[:, 0, 0::2, 0::2]   # R(e,e)
    out[:, 1, 0::2, 1::2] = x[:, 0, 0::2, 1::2]   # G(e,o)
    out[:, 1, 1::2, 0::2] = x[:, 0, 1::2, 0::2]   # G(o,e)
    out[:, 2, 1::2, 1::2] = x[:, 0, 1::2, 1::2]   # B(o,o)
    yv = y.reshape(Bn, NQ, NB, NS, 8, QR, QC)
    for q in range(NQ):
        for s in range(NS):
            c0 = 480 * q + SW * s
            sub = yv[:, q, :, s]                   # (B, NB, 8, QR, QC)
            for pl, ch, rp, cp in PLANES:
                arr = sub[:, :, pl].reshape(Bn, NB * QR, QC)
                out[:, ch, rp::2, c0 + cp:c0 + SW:2] = arr
    return out


def kernel(x, kernels=None, index=None, **_unused):
    global LAST_RESULTS
    x = np.ascontiguousarray(np.asarray(x), dtype=np.float32)
    Bn = x.shape[0]
    xprep = _prep_inputs(x)
    nc = _build()
    from concourse.bass_utils import run_bass_kernel_spmd
    in_maps = [{"xprep": xprep[i]} for i in range(Bn)]
    res = run_bass_kernel_spmd(nc, in_maps, core_ids=list(range(Bn)))
    LAST_RESULTS = res
    y = np.stack([res.results[i]["yout"] for i in range(Bn)])
    return _assemble(y, x)


# revision 5
# speedup vs baseline: 2.0163x; 2.0163x over previous
"""Debayer 3x3 kernel for Trainium2 (Bass/Tile), batch-sharded over 8 NeuronCores.

Reference semantics: 1->5 channel 3x3 conv (identity, plus-4, diag-4,
horiz-2, vert-2) over an edge-padded Bayer frame, then per-2x2-parity
channel select into RGB.

v3 (memory-optimized): device input is fp16 pre-scaled by S=63.875 on
the host (so no on-device prescale pass), device output is uint8
fixed-point (tolerance is 2e-2; quantization adds <5e-3). The identity
channel (1 of every 3 output values equals the input pixel exactly) is
pasted on the host from the original f32 input. The device computes the
8 non-trivial quarter-resolution planes per tile, packed contiguously
for one DMA per slice:
  P0 c1_ee->G  P1 c1_oo->G  P2 c2_ee->B  P3 c2_oo->R
  P4 c3_eo->R  P5 c3_oe->B  P6 c4_eo->B  P7 c4_oe->R
With Q = S*x:
  SQ[r,c] = Q[r,c]+Q[r,c+2]   (horiz pair, centered at out col c)
  VQ[r,c] = Q[r,c]+Q[r+2,c]   (vert pair, centered at out row r)
  c1*4S = SQ+VQ   c2*4S = SQ[up]+SQ[down]   c3*4S = 2*SQ   c4*4S = 2*VQ
All plane values < 4S < 255.5, so the u8 convert cannot overflow even
with round-to-nearest. Host unpack divides by 4S.

Device traffic per core: in 128*4*36*122*2B = 4.5 MB, out
128*4*8*17*60*1B = 2.1 MB (vs 34 MB for the f32 3-channel baseline).
Full-res pair sums run on DVE in 2x packed-fp16 mode (step-1, 4B-aligned
APs); the stride-2 parity combines run 1x wherever they live — engine
choice per pair is a knob (HW showed gpsimd contends with DVE's SBUF
port, so default is DVE + act).

Device layout: the host pre-tiles each padded 1090x1922 fp16 image into
128 partitions x 4 col-slices x (36 rows x 122 cols) patches:
  partition p = 32*q + b  (col-quarter q in 0..3, row-band b in 0..31)
  band b   -> image rows [34b, 34b+34)        (patch has +-1 halo rows)
  slice s  -> image cols [480q+120s, +120)    (patch has +-1 halo cols)
34 and 120 are even so parity phase is uniform across partitions/slices.
"""

import numpy as np

H, W = 1088, 1920
NB = 32          # row bands per column-quarter
BH = 34          # output rows per band
NQ = 4           # column quarters
NS = 4           # col slices per patch
SW = 120         # output cols per slice
PR, PC = BH + 2, SW + 2   # patch rows/cols (with halo)
QR, QC = 17, 60           # quarter-res plane dims per tile
SCALE = 63.875            # host prescale; plane fixed-point = 4*SCALE*value

# (plane, channel, row parity, col parity) for host-side assembly
PLANES = [
    (0, 1, 0, 0),  # c1_ee -> G
    (1, 1, 1, 1),  # c1_oo -> G
    (2, 2, 0, 0),  # c2_ee -> B
    (3, 0, 1, 1),  # c2_oo -> R
    (4, 0, 0, 1),  # c3_eo -> R
    (5, 2, 1, 0),  # c3_oe -> B
    (6, 2, 0, 1),  # c4_eo -> B
    (7, 0, 1, 0),  # c4_oe -> R
]

_NC_CACHE = {}
LAST_RESULTS = None


def _build(reps=1, *, e_c1="vector", e_c2="vector", in_bufs=3, mid_bufs=2,
           out_bufs=2, skeleton=False, **_ignored):
    """Build the Bass module. reps>1 repeats the whole pipeline (bench only:
    amortizes per-dispatch overhead out of wall-clock measurements)."""
    key = (reps, e_c1, e_c2, in_bufs, mid_bufs, out_bufs, skeleton)
    if key in _NC_CACHE:
        return _NC_CACHE[key]
    import concourse.bacc as bacc
    import concourse.mybir as mybir
    import concourse.tile as tile
    from concourse._compat import get_trn_type

    f16 = mybir.dt.float16
    u8 = mybir.dt.uint8
    nc = bacc.Bacc(get_trn_type() or "TRN2", target_bir_lowering=False, debug=False)
    xin = nc.dram_tensor("xprep", [128, NS, PR, PC], f16, kind="ExternalInput")
    yout = nc.dram_tensor("yout", [128, NS, 8, QR, QC], u8, kind="ExternalOutput")
    # bench-only: earlier reps dump to internal scratch so no two reps write
    # the same DRAM (WAW races hang the exec unit)
    ydumps = [
        nc.dram_tensor(f"ydump{r}", [128, NS, 8, QR, QC], u8, kind="Internal")
        for r in range(reps - 1)
    ]

    with tile.TileContext(nc) as tc:
        with tc.tile_pool(name="pin", bufs=in_bufs) as pin, \
             tc.tile_pool(name="pmid", bufs=mid_bufs) as pmid, \
             tc.tile_pool(name="pout", bufs=out_bufs) as pout:

            eng1 = {"gpsimd": nc.gpsimd, "vector": nc.vector}[e_c1]
            eng2 = {"gpsimd": nc.gpsimd, "vector": nc.vector}[e_c2]

            def load(j):
                t = pin.tile([128, PR, PC], f16, tag="inp", name=f"inp{j}")
                nc.sync.dma_start(out=t[:], in_=xin[:, j % NS])
                return t

            cur = load(0)
            for j in range(NS * reps):
                k = j % NS
                r = j // NS
                ytgt = yout if r == reps - 1 else ydumps[r]
                nxt = load(j + 1) if j + 1 < NS * reps else None
                Q = cur
                Y = pout.tile([128, 8, QR, QC], u8, tag="y", name=f"y{k}")
                if skeleton:
                    # bench-only: DMA floor (touch input once so it's live)
                    nc.vector.tensor_copy(Y[:, 0, 0:1, :], Q[:, 0:1, 0:QC])
                    nc.sync.dma_start(out=ytgt[:, k], in_=Y[:])
                    cur = nxt
                    continue
                # SQ[p, r, c] = S*(l+r) centered at out col c (TT, 2x mode)
                SQ = pmid.tile([128, PR, SW], f16, tag="sq", name=f"sq{k}")
                nc.vector.tensor_add(SQ[:], Q[:, :, 0:SW], Q[:, :, 2:PC])
                # VQ[p, r, c] = S*(u+d) centered at out row r, full patch width
                VQ = pmid.tile([128, PR - 2, PC], f16, tag="vq", name=f"vq{k}")
                nc.vector.tensor_add(VQ[:], Q[:, 0:PR - 2, :], Q[:, 2:PR, :])

                # c1 = SQ+VQ, c2 = SQ up+down (stride-2 parity reads, u8 out)
                eng1.tensor_add(Y[:, 0], SQ[:, 1:35:2, 0:120:2], VQ[:, 0:34:2, 1:121:2])
                eng1.tensor_add(Y[:, 1], SQ[:, 2:36:2, 1:120:2], VQ[:, 1:34:2, 2:122:2])
                eng2.tensor_add(Y[:, 2], SQ[:, 0:34:2, 0:120:2], SQ[:, 2:36:2, 0:120:2])
                eng2.tensor_add(Y[:, 3], SQ[:, 1:35:2, 1:120:2], SQ[:, 3:36:2, 1:120:2])
                # c3 = 2*SQ, c4 = 2*VQ (scaled copies on the act engine, u8 out)
                nc.scalar.mul(Y[:, 4], SQ[:, 1:35:2, 1:120:2], 2.0)
                nc.scalar.mul(Y[:, 5], SQ[:, 2:36:2, 0:120:2], 2.0)
                nc.scalar.mul(Y[:, 6], VQ[:, 0:34:2, 2:122:2], 2.0)
                nc.scalar.mul(Y[:, 7], VQ[:, 1:34:2, 1:121:2], 2.0)
                nc.sync.dma_start(out=ytgt[:, k], in_=Y[:])

                cur = nxt

    nc.compile()
    _NC_CACHE[key] = nc
    return nc


def _prep_inputs(x):
    """(B,1,1088,1920) f32 -> (B,128,NS,PR,PC) fp16 SCALE*x patches."""
    Bn = x.shape[0]
    xpad = np.pad(x[:, 0], ((0, 0), (1, 1), (1, 1)), mode="edge")
    xpad = (xpad * np.float32(SCALE)).astype(np.float16)
    xprep = np.empty((Bn, 128, NS, PR, PC), np.float16)
    st = xpad.strides
    for q in range(NQ):
        for s in range(NS):
            c0 = 480 * q + SW * s
            block = xpad[:, :, c0:c0 + PC]
            v = np.lib.stride_tricks.as_strided(
                block, shape=(Bn, NB, PR, PC),
                strides=(st[0], BH * st[1], st[1], st[2]))
            xprep[:, q * NB:(q + 1) * NB, s] = v
    return xprep


# u8 unpack: value = (y + UNPACK_BIAS) / (4*SCALE); bias 0.5 assumes the
# f16->u8 convert truncates (empirical; set to 0.0 if it rounds)
UNPACK_BIAS = 0.5


def _assemble(y, x):
    """y (B,128,NS,8,QR,QC) u8 planes + x (B,1,H,W) f32 -> (B,3,H,W) f32."""
    Bn = x.shape[0]
    out = np.empty((Bn, 3, H, W), np.float32)
    # identity channels from the exact f32 input
    out[:, 0, 0::2, 0::2] = x[:, 0, 0::2, 0::2]   # R(e,e)
    out[:, 1, 0::2, 1::2] = x[:, 0, 0::2, 1::2]   # G(e,o)
    out[:, 1, 1::2, 0::2] = x[:, 0, 1::2, 0::2]   # G(o,e)
    out[:, 2, 1::2, 1::2] = x[:, 0, 1::2, 1::2]   # B(o,o)
    inv = np.float32(1.0 / (4.0 * SCALE))
    yf = (y.astype(np.float32) + np.float32(UNPACK_BIAS)) * inv
    yv = yf.reshape(Bn, NQ, NB, NS, 8, QR, QC)
    for q in range(NQ):
        for s in range(NS):
            c0 = 480 * q + SW * s
            sub = yv[:, q, :, s]                   # (B, NB, 8, QR, QC)
            for pl, ch, rp, cp in PLANES:
                arr = sub[:, :, pl].reshape(Bn, NB * QR, QC)
                out[:, ch, rp::2, c0 + cp:c0 + SW:2] = arr
    return out


def kernel(x, kernels=None, index=None, **_unused):
    global LAST_RESULTS
    x = np.ascontiguousarray(np.asarray(x), dtype=np.float32)
    Bn = x.shape[0]
    xprep = _prep_inputs(x)
    nc = _build()
    from concourse.bass_utils import run_bass_kernel_spmd
    in_maps = [{"xprep": xprep[i]} for i in range(Bn)]
    res = run_bass_kernel_spmd(nc, in_maps, core_ids=list(range(Bn)))
    LAST_RESULTS = res
    y = np.stack([res.results[i]["yout"] for i in range(Bn)])
    return _assemble(y, x)


# revision 12
# speedup vs baseline: 2.5642x; 1.2718x over previous
"""Debayer 3x3 kernel for Trainium2 (Bass/Tile), batch-sharded over 8 NeuronCores.

Reference semantics: 1->5 channel 3x3 conv (identity, plus-4, diag-4,
horiz-2, vert-2) over an edge-padded Bayer frame, then per-2x2-parity
channel select into RGB.

v3 (memory-optimized): device input is fp16 pre-scaled by S=63.875 on
the host (so no on-device prescale pass), device output is uint8
fixed-point (tolerance is 2e-2; quantization adds <5e-3). The identity
channel (1 of every 3 output values equals the input pixel exactly) is
pasted on the host from the original f32 input. The device computes the
8 non-trivial quarter-resolution planes per tile, packed contiguously
for one DMA per slice:
  P0 c1_ee->G  P1 c1_oo->G  P2 c2_ee->B  P3 c2_oo->R
  P4 c3_eo->R  P5 c3_oe->B  P6 c4_eo->B  P7 c4_oe->R
With Q = S*x:
  SQ[r,c] = Q[r,c]+Q[r,c+2]   (horiz pair, centered at out col c)
  VQ[r,c] = Q[r,c]+Q[r+2,c]   (vert pair, centered at out row r)
  c1*4S = SQ+VQ   c2*4S = SQ[up]+SQ[down]   c3*4S = 2*SQ   c4*4S = 2*VQ
All plane values < 4S < 255.5, so the u8 convert cannot overflow even
with round-to-nearest. Host unpack divides by 4S.

Device traffic per core: in 128*4*36*122*2B = 4.5 MB, out
128*4*8*17*60*1B = 2.1 MB (vs 34 MB for the f32 3-channel baseline).
Full-res pair sums run on DVE in 2x packed-fp16 mode (step-1, 4B-aligned
APs); the stride-2 parity combines run 1x wherever they live — engine
choice per pair is a knob (HW showed gpsimd contends with DVE's SBUF
port, so default is DVE + act).

Device layout: the host pre-tiles each padded 1090x1922 fp16 image into
128 partitions x 4 col-slices x (36 rows x 122 cols) patches:
  partition p = 32*q + b  (col-quarter q in 0..3, row-band b in 0..31)
  band b   -> image rows [34b, 34b+34)        (patch has +-1 halo rows)
  slice s  -> image cols [480q+120s, +120)    (patch has +-1 halo cols)
34 and 120 are even so parity phase is uniform across partitions/slices.
"""

import numpy as np

H, W = 1088, 1920
NB = 32          # row bands per column-quarter
BH = 34          # output rows per band
NQ = 4           # column quarters
NS = 4           # col slices per patch
SW = 120         # output cols per slice
PR, PC = BH + 2, SW + 2   # patch rows/cols (with halo)
QR, QC = 17, 60           # quarter-res plane dims per tile
SCALE = 63.875            # host prescale; plane fixed-point = 4*SCALE*value

# (plane, channel, row parity, col parity) for host-side assembly
PLANES = [
    (0, 1, 0, 0),  # c1_ee -> G
    (1, 1, 1, 1),  # c1_oo -> G
    (2, 2, 0, 0),  # c2_ee -> B
    (3, 0, 1, 1),  # c2_oo -> R
    (4, 0, 0, 1),  # c3_eo -> R
    (5, 2, 1, 0),  # c3_oe -> B
    (6, 2, 0, 1),  # c4_eo -> B
    (7, 0, 1, 0),  # c4_oe -> R
]

_NC_CACHE = {}
LAST_RESULTS = None


def _build(reps=1, *, e_c1="vector", e_c2="vector", in_bufs=3, mid_bufs=2,
           out_bufs=2, skeleton=False, dma_in="sync", dma_out="sync",
           **_ignored):
    """Build the Bass module. reps>1 repeats the whole pipeline (bench only:
    amortizes per-dispatch overhead out of wall-clock measurements).
    dma_in/dma_out: "sync" | "scalar" | "alt"/"alt2" (alternate rings by
    slice; alt2 is the opposite phase) — HWDGE has two physical rings."""
    key = (reps, e_c1, e_c2, in_bufs, mid_bufs, out_bufs, skeleton,
           dma_in, dma_out)
    # e_c1="gpsimd16": c1 adds on gpsimd in fp16 (it cannot write u8),
    # act converts to u8; act absorbs it since it is far from its roofline
    if key in _NC_CACHE:
        return _NC_CACHE[key]
    import concourse.bacc as bacc
    import concourse.mybir as mybir
    import concourse.tile as tile
    from concourse._compat import get_trn_type

    f16 = mybir.dt.float16
    u8 = mybir.dt.uint8
    nc = bacc.Bacc(get_trn_type() or "TRN2", target_bir_lowering=False, debug=False)
    xin = nc.dram_tensor("xprep", [128, NS, PR, PC], f16, kind="ExternalInput")
    yout = nc.dram_tensor("yout", [128, NS, 8, QR, QC], u8, kind="ExternalOutput")
    # bench-only: earlier reps dump to internal scratch so no two reps write
    # the same DRAM (WAW races hang the exec unit)
    ydumps = [
        nc.dram_tensor(f"ydump{r}", [128, NS, 8, QR, QC], u8, kind="Internal")
        for r in range(reps - 1)
    ]

    with tile.TileContext(nc) as tc:
        with tc.tile_pool(name="pin", bufs=in_bufs) as pin, \
             tc.tile_pool(name="pmid", bufs=mid_bufs) as pmid, \
             tc.tile_pool(name="pout", bufs=out_bufs) as pout:

            eng1 = {"gpsimd": nc.gpsimd, "vector": nc.vector,
                    "gpsimd16": None}[e_c1]
            eng2 = {"gpsimd": nc.gpsimd, "vector": nc.vector}[e_c2]

            def ring(which, j):
                if which == "alt":
                    return nc.sync if j % 2 == 0 else nc.scalar
                if which == "alt2":
                    return nc.scalar if j % 2 == 0 else nc.sync
                return {"sync": nc.sync, "scalar": nc.scalar}[which]

            def load(j):
                t = pin.tile([128, PR, PC], f16, tag="inp", name=f"inp{j}")
                ring(dma_in, j).dma_start(out=t[:], in_=xin[:, j % NS])
                return t

            cur = load(0)
            for j in range(NS * reps):
                k = j % NS
                r = j // NS
                ytgt = yout if r == reps - 1 else ydumps[r]
                nxt = load(j + 1) if j + 1 < NS * reps else None
                Q = cur
                Y = pout.tile([128, 8, QR, QC], u8, tag="y", name=f"y{k}")
                if skeleton:
                    # bench-only: DMA floor (touch input once so it's live)
                    nc.vector.tensor_copy(Y[:, 0, 0:1, :], Q[:, 0:1, 0:QC])
                    ring(dma_out, j).dma_start(out=ytgt[:, k], in_=Y[:])
                    cur = nxt
                    continue
                # SQ[p, r, c] = S*(l+r) centered at out col c (TT, 2x mode)
                SQ = pmid.tile([128, PR, SW], f16, tag="sq", name=f"sq{k}")
                nc.vector.tensor_add(SQ[:], Q[:, :, 0:SW], Q[:, :, 2:PC])
                # VQ[p, r, c] = S*(u+d) centered at out row r, full patch width
                VQ = pmid.tile([128, PR - 2, PC], f16, tag="vq", name=f"vq{k}")
                nc.vector.tensor_add(VQ[:], Q[:, 0:PR - 2, :], Q[:, 2:PR, :])

                # c1 = SQ+VQ, c2 = SQ up+down (stride-2 parity reads, u8 out)
                if e_c1 == "gpsimd16":
                    C1 = pmid.tile([128, 2, QR, QC], f16, tag="c1", name=f"c1{k}")
                    nc.gpsimd.tensor_add(C1[:, 0], SQ[:, 1:35:2, 0:120:2], VQ[:, 0:34:2, 1:121:2])
                    nc.gpsimd.tensor_add(C1[:, 1], SQ[:, 2:36:2, 1:120:2], VQ[:, 1:34:2, 2:122:2])
                    nc.scalar.mul(Y[:, 0], C1[:, 0], 1.0)
                    nc.scalar.mul(Y[:, 1], C1[:, 1], 1.0)
                else:
                    eng1.tensor_add(Y[:, 0], SQ[:, 1:35:2, 0:120:2], VQ[:, 0:34:2, 1:121:2])
                    eng1.tensor_add(Y[:, 1], SQ[:, 2:36:2, 1:120:2], VQ[:, 1:34:2, 2:122:2])
                eng2.tensor_add(Y[:, 2], SQ[:, 0:34:2, 0:120:2], SQ[:, 2:36:2, 0:120:2])
                eng2.tensor_add(Y[:, 3], SQ[:, 1:35:2, 1:120:2], SQ[:, 3:36:2, 1:120:2])
                # c3 = 2*SQ, c4 = 2*VQ (scaled copies on the act engine, u8 out)
                nc.scalar.mul(Y[:, 4], SQ[:, 1:35:2, 1:120:2], 2.0)
                nc.scalar.mul(Y[:, 5], SQ[:, 2:36:2, 0:120:2], 2.0)
                nc.scalar.mul(Y[:, 6], VQ[:, 0:34:2, 2:122:2], 2.0)
                nc.scalar.mul(Y[:, 7], VQ[:, 1:34:2, 1:121:2], 2.0)
                ring(dma_out, j).dma_start(out=ytgt[:, k], in_=Y[:])

                cur = nxt

    nc.compile()
    _NC_CACHE[key] = nc
    return nc


def _prep_inputs(x):
    """(B,1,1088,1920) f32 -> (B,128,NS,PR,PC) fp16 SCALE*x patches."""
    Bn = x.shape[0]
    xpad = np.pad(x[:, 0], ((0, 0), (1, 1), (1, 1)), mode="edge")
    xpad = (xpad * np.float32(SCALE)).astype(np.float16)
    xprep = np.empty((Bn, 128, NS, PR, PC), np.float16)
    st = xpad.strides
    for q in range(NQ):
        for s in range(NS):
            c0 = 480 * q + SW * s
            block = xpad[:, :, c0:c0 + PC]
            v = np.lib.stride_tricks.as_strided(
                block, shape=(Bn, NB, PR, PC),
                strides=(st[0], BH * st[1], st[1], st[2]))
            xprep[:, q * NB:(q + 1) * NB, s] = v
    return xprep


# u8 unpack: value = (y + UNPACK_BIAS) / (4*SCALE); bias 0.5 assumes the
# f16->u8 convert truncates (empirical; set to 0.0 if it rounds)
UNPACK_BIAS = 0.5


def _assemble(y, x):
    """y (B,128,NS,8,QR,QC) u8 planes + x (B,1,H,W) f32 -> (B,3,H,W) f32."""
    Bn = x.shape[0]
    out = np.empty((Bn, 3, H, W), np.float32)
    # identity channels from the exact f32 input
    out[:, 0, 0::2, 0::2] = x[:, 0, 0::2, 0::2]   # R(e,e)
    out[:, 1, 0::2, 1::2] = x[:, 0, 0::2, 1::2]   # G(e,o)
    out[:, 1, 1::2, 0::2] = x[:, 0, 1::2, 0::2]   # G(o,e)
    out[:, 2, 1::2, 1::2] = x[:, 0, 1::2, 1::2]   # B(o,o)
    inv = np.float32(1.0 / (4.0 * SCALE))
    yf = (y.astype(np.float32) + np.float32(UNPACK_BIAS)) * inv
    yv = yf.reshape(Bn, NQ, NB, NS, 8, QR, QC)
    for q in range(NQ):
        for s in range(NS):
            c0 = 480 * q + SW * s
            sub = yv[:, q, :, s]                   # (B, NB, 8, QR, QC)
            for pl, ch, rp, cp in PLANES:
                arr = sub[:, :, pl].reshape(Bn, NB * QR, QC)
                out[:, ch, rp::2, c0 + cp:c0 + SW:2] = arr
    return out


def kernel(x, kernels=None, index=None, **_unused):
    global LAST_RESULTS
    x = np.ascontiguousarray(np.asarray(x), dtype=np.float32)
    Bn = x.shape[0]
    xprep = _prep_inputs(x)
    nc = _build()
    from concourse.bass_utils import run_bass_kernel_spmd
    in_maps = [{"xprep": xprep[i]} for i in range(Bn)]
    res = run_bass_kernel_spmd(nc, in_maps, core_ids=list(range(Bn)))
    LAST_RESULTS = res
    y = np.stack([res.results[i]["yout"] for i in range(Bn)])
    return _assemble(y, x)
